# revision 24
# baseline (speedup 1.0000x reference)
"""SATD loss kernel for Trainium2: sum |H @ (original - pred)|.

Full inputs: original, pred [2, 8192, 64, 64] f32. H is the 64x64
Sylvester Hadamard matrix applied along axis -2 of each 64x64 block.

Strategy (8-way data parallel over the 16384 blocks, 2048 per core):
  - Host: shard blocks across cores, cast to bf16 (H has +-1 entries;
    the transform accumulates in fp32 PSUM, and the bf16 input rounding
    contributes ~1e-6 relative error on the final scalar), and repack
    each core's data into [T, 128, 2*COLS] tiles whose partition axis
    holds the j-rows of 128 blocks (two 64-block halves m=0/1 on
    partitions 0-63 / 64-127) and whose free axis is (g, k) for
    original then pred.
  - Device, per tile: one contiguous 4 MiB DMA; per 512-column slice,
    matmul with lhsT = kron(I2, H) on the original half, then
    accumulate matmul with -kron(I2, H) on the pred half into the same
    PSUM bank -> PSUM = H @ (A - B) for 16 blocks at 128 partitions.
  - Fused abs+sum (tensor_reduce apply_absolute_value on VectorE) per
    PSUM bank into an SBUF accumulator; final reduce -> [128, 1]/core.
  - Host sums the 8x128 partials (f64) and casts to f32.
"""

from contextlib import ExitStack

import ml_dtypes
import numpy as np

import concourse.bass as bass
import concourse.tile as tile
from concourse import bacc, mybir
from concourse.bass_utils import run_bass_kernel_spmd

N_CORES = 8
N = 64                       # Hadamard block size
BLOCKS_TOTAL = 2 * 8192      # 16384 blocks of [64, 64]
BLOCKS_PER_CORE = BLOCKS_TOTAL // N_CORES   # 2048
G = 64                       # blocks per partition-half per tile
COLS = G * N                 # 4096 bf16 = 8 KiB per partition per input
TILES = BLOCKS_PER_CORE // (2 * G)          # 16 iterations
MM_N = 512                   # matmul moving free dim (one PSUM bank)
SUB = COLS // MM_N           # psum tiles per SBUF tile (8)

F32 = mybir.dt.float32
# Input quantization: "bf16" (rel err ~1e-6) or "e4m3" (rel err ~4e-4,
# half the DMA traffic). PSUM accumulation is fp32 either way.
import os
QUANT = os.environ.get("SATD_QUANT", "e4m3")
if QUANT == "e4m3":
    IN_DT = mybir.dt.float8e4
    IN_NP = ml_dtypes.float8_e4m3
else:
    IN_DT = mybir.dt.bfloat16
    IN_NP = ml_dtypes.bfloat16


def _hadamard(n: int) -> np.ndarray:
    H = np.array([[1.0]], dtype=np.float32)
    while H.shape[0] < n:
        H = np.block([[H, H], [H, -H]])
    return H.astype(np.float32)


def _weights() -> np.ndarray:
    # lhsT for out = Hd @ rhs is Hd.T; kron(I2, H) is symmetric.
    Hd = np.kron(np.eye(2, dtype=np.float32), _hadamard(N))
    return np.concatenate([Hd, -Hd], axis=1).astype(
        IN_NP)  # [128, 256], entries +-1 exact in bf16/e4m3


def _build_program() -> bacc.Bacc:
    nc = bacc.Bacc("TRN2", target_bir_lowering=False, debug=False,
                   num_devices=N_CORES)
    x = nc.dram_tensor("x", [TILES, 128, 2 * COLS], IN_DT,
                       kind="ExternalInput").ap()
    w = nc.dram_tensor("w", [128, 256], IN_DT, kind="ExternalInput").ap()
    out = nc.dram_tensor("out", [128, 2], F32, kind="ExternalOutput").ap()

    with tile.TileContext(nc) as tc, ExitStack() as ctx:
        wpool = ctx.enter_context(tc.tile_pool(name="w", bufs=1))
        xpool = ctx.enter_context(tc.tile_pool(name="x", bufs=3))
        psum = ctx.enter_context(tc.tile_pool(name="psum", bufs=2,
                                              space="PSUM"))
        accpool = ctx.enter_context(tc.tile_pool(name="acc", bufs=1))
        scratch = ctx.enter_context(tc.tile_pool(name="scr", bufs=2))

        wt = wpool.tile([128, 256], IN_DT)
        nc.sync.dma_start(wt[:], w[:])
        w_pos = wt[:, 0:128]
        w_neg = wt[:, 128:256]

        # Separate accumulators per reduce engine so VectorE and ScalarE
        # never touch the same tile (no cross-engine serialization).
        ncols = TILES
        accv = accpool.tile([128, ncols], F32, tag="accv")
        acca = accpool.tile([128, ncols], F32, tag="acca")

        w3 = wt[:].rearrange("p (h m) -> p h m", h=2)

        for t in range(TILES):
            xt = xpool.tile([128, 2 * COLS], IN_DT)
            nc.sync.dma_start(xt[:, :], x[t])
            # [p, h, c]: h=0 selects the original half, h=1 the pred
            # half; DoubleRow contracts over (p, h) in one pass, so a
            # single matmul computes Hd@A - Hd@B per 512-column slice.
            x3 = xt[:].rearrange("p (h c) -> p h c", h=2)
            # Two 4-bank PSUM tiles per iteration; one wide abs+sum per
            # tile (VectorE for the first, ScalarE for the second) to
            # amortize per-op overheads.
            for half in range(2):
                pt = psum.tile([128, 4 * MM_N], F32)
                for q in range(4):
                    s = half * 4 + q
                    nc.tensor.matmul(
                        pt[:, q * MM_N:(q + 1) * MM_N], w3,
                        x3[:, :, s * MM_N:(s + 1) * MM_N],
                        start=True, stop=True,
                        perf_mode=mybir.MatmulPerfMode.DoubleRow)
                if half == 0:
                    nc.vector.tensor_reduce(
                        accv[:, t:t + 1], pt[:],
                        axis=mybir.AxisListType.X, op=mybir.AluOpType.add,
                        apply_absolute_value=True)
                else:
                    st = scratch.tile([128, 4 * MM_N], F32)
                    nc.scalar.activation(
                        st[:], pt[:], mybir.ActivationFunctionType.Abs,
                        accum_out=acca[:, t:t + 1])

        res = accpool.tile([128, 2], F32, tag="res")
        nc.vector.tensor_reduce(res[:, 0:1], accv[:],
                                axis=mybir.AxisListType.X,
                                op=mybir.AluOpType.add)
        nc.vector.tensor_reduce(res[:, 1:2], acca[:],
                                axis=mybir.AxisListType.X,
                                op=mybir.AluOpType.add)
        nc.sync.dma_start(out[:], res[:])

    nc.compile()
    return nc


def _repack(shard: np.ndarray) -> np.ndarray:
    """[BLOCKS_PER_CORE, 64, 64] bf16 -> [TILES, 128, COLS] with
    partition axis (m, j) and free axis (g, k)."""
    v = shard.reshape(TILES, 2, G, N, N)          # t, m, g, j, k
    v = v.transpose(0, 1, 3, 2, 4)                # t, m, j, g, k
    return v.reshape(TILES, 128, COLS)


_NC = None


def _get_program() -> bacc.Bacc:
    global _NC
    if _NC is None:
        _NC = _build_program()
    return _NC


def _run(original: np.ndarray, pred: np.ndarray, **spmd_kwargs):
    a_full = np.asarray(original, dtype=np.float32).reshape(
        BLOCKS_TOTAL, N, N).astype(IN_NP)
    b_full = np.asarray(pred, dtype=np.float32).reshape(
        BLOCKS_TOTAL, N, N).astype(IN_NP)
    wnp = _weights()
    in_maps = []
    for i in range(N_CORES):
        sl = slice(i * BLOCKS_PER_CORE, (i + 1) * BLOCKS_PER_CORE)
        xi = np.empty((TILES, 128, 2 * COLS), dtype=IN_NP)
        xi[:, :, :COLS] = _repack(a_full[sl])
        xi[:, :, COLS:] = _repack(b_full[sl])
        in_maps.append({"x": xi, "w": wnp})
    nc = _get_program()
    r = run_bass_kernel_spmd(nc, in_maps, list(range(N_CORES)),
                             **spmd_kwargs)
    total = 0.0
    for i in range(N_CORES):
        total += r.results[i]["out"].astype(np.float64).sum()
    return np.float32(total), r


def kernel(original: np.ndarray, pred: np.ndarray) -> np.ndarray:
    val, _ = _run(original, pred)
    return np.array(val, dtype=np.float32)
